# revision 27
# baseline (speedup 1.0000x reference)
"""Concordance-index (C-index) kernel for Trainium2, 8 NeuronCores — v4.2.

Math
----
Reference computes, over all pairs i<j of N=16384 samples:
    cc = ((y_i>=y_j & yh_i>=yh_j & st_j) | (y_i<=y_j & yh_i<=yh_j & st_i)) & triu
    tp = ((y_i<=y_j & st_i) | (y_i>=y_j & st_j)) & triu
    out = sum(cc) / sum(tp)

With A_ae = [y_a >= y_e], B_ae = [yh_a >= yh_e] over (a = all, e = event):
    sum(cc) = S1 - ns - OV,   S1 = sum_e T_e,  T_e = sum_a A_ae * B_ae
    sum(tp) = S2 - ns - OV2,  S2 = sum_e sum_a A_ae
where OV / OV2 are exact-fp32-tie corrections among event pairs.

Key ideas
---------
* Sort the a-axis by fp32 y: A_ae = [a >= c_e], c_e host-exact, so S2 is
  pure host math and T_e is a 1-D count of [yh_a >= t_e] over the suffix
  [c_e, N) — or R_e (host-known total) minus the prefix count.  Each
  event pays only its SHORTER side: ~N^2/8 compares total.
* Events sorted by c, 1024 per row (128/core x 8 cores).  The ragged
  per-event boundary is absorbed into a HOST-GATHERED window per event
  (padded with -60000 -> compares contribute 0); the rest is a
  row-uniform 64-aligned tail.  Every device op is a plain fused
  compare+accumulate — no masks, no products.
* S1 only needs GLOBAL sums, so three lanes run concurrently:
  ACT (Sign+accum), DVE (is_ge+add fused), and DVE plain is_ge in 4x
  DVE perf mode feeding PE ones-matmul column reductions in PSUM.
* DMA is chunked over three queue rings (sync/gpsimd/tensor) in the
  order ops consume it; ops are emitted in estimated-arrival order.
* fp16 compare semantics host+device; only fp16-vs-fp32 tie noise
  remains (~1e-4 rel; gate 2e-2).
"""

import math
import os
import sys

import numpy as np

for _p in ("/opt/trn_rl_repo", "/root/.axon_site", "/root/.axon_site/_ro/trn_rl_repo"):
    if os.path.isdir(_p) and _p not in sys.path:
        sys.path.append(_p)

import ml_dtypes  # noqa: F401  (env check)

import concourse.bacc as bacc
import concourse.mybir as mybir
from concourse import bass_utils
from concourse import tile

N = 16384
P = 128
NCORES = 8
NEG16 = np.float16(-60000.0)
BIG32 = np.float32(1e30)
TCH = 3072          # tail/bcast chunk width
MM = 512            # PE matmul moving width (one PSUM bank)

FP32 = mybir.dt.float32
FP16 = mybir.dt.float16
BF16 = mybir.dt.bfloat16
Alu = mybir.AluOpType
ActF = mybir.ActivationFunctionType

# measured ns/elem and ns/op overhead (TRN2, aligned ops)
COST = {
    "dve": (1.09, 240.0),    # tensor_scalar is_ge+add fused accum (1x)
    "act": (0.95, 600.0),    # activation Sign with accum (big read-acc cost)
    "pe": (0.29, 130.0),     # DVE plain is_ge at 4x (feeds PE)
}
ACT_MIN = 2048               # ACT lane only for ops at least this wide
PE_RATE = 1.05               # ns/elem ones-matmul reduce incl ldweights
DMA_NAT = 6.1e-3             # ns/SBUF-byte, natural DMA (~163 GB/s, HBM-read bound)
DMA_BC = 1.9e-3              # ns/SBUF-byte, broadcast DMA (~530 GB/s)


def _mid32(t16):
    prev = np.nextafter(t16, np.float16(-np.inf))
    return (t16.astype(np.float32) + prev.astype(np.float32)) * np.float32(0.5)


class Plan:
    def __init__(self, y, yh, status):
        y32 = np.asarray(y, np.float32)
        yh32 = np.asarray(yh, np.float32)
        st = np.asarray(status)

        order = np.argsort(y32, kind="stable")
        ys = y32[order]
        self.yh16 = yh32[order].astype(np.float16)

        ev = np.nonzero(st == 1)[0]
        self.ns = int(len(ev))
        t16 = yh32[ev].astype(np.float16)
        c = np.searchsorted(ys, y32[ev], side="left").astype(np.int64)
        yh16s = np.sort(self.yh16)
        R = (N - np.searchsorted(yh16s, t16, side="left")).astype(np.int64)
        self.S2 = int((np.int64(N) - c).sum())

        yv = y32[ev]
        _, cnt = np.unique(yv, return_counts=True)
        self.OV2 = int((cnt.astype(np.int64) * (cnt - 1) // 2).sum())
        pair = np.stack([yv, yh32[ev]], 1)
        _, cnt2 = np.unique(pair, axis=0, return_counts=True)
        self.OV = int((cnt2.astype(np.int64) * (cnt2 - 1) // 2).sum())

        eo = np.argsort(c, kind="stable")
        self.c = c[eo]
        self.t16 = t16[eo]
        self.R = R[eo]

        self.nrow = max(1, math.ceil(self.ns / (P * NCORES)))
        self.rows = []
        for r in range(self.nrow):
            s, e = r * P * NCORES, min((r + 1) * P * NCORES, self.ns)
            cr = self.c[s:e]
            lo, hi = int(cr.min()), int(cr.max())
            suffix = int((np.int64(N) - cr).sum()) <= int(cr.sum())
            if suffix:
                a = min(N, ((hi + 1 + MM - 1) // MM) * MM)  # 512-aligned
                tail = (a, N)
                sel = [j for j in range((lo + MM - 1) // MM, a // MM)]
            else:
                b = (lo // MM) * MM
                tail = (0, b)
                sel = [j for j in range(b // MM, N // MM)
                       if MM * (j + 1) <= hi]
            self.rows.append({"suffix": suffix, "s": s, "e": e, "lo": lo,
                              "hi": hi, "wh": MM, "tail": tail, "sel": sel,
                              "abound": a if suffix else b})

        # ---- broadcast ranges: tails + selective blocks, merged ----
        cover = [row["tail"] for row in self.rows
                 if row["tail"][1] > row["tail"][0]]
        for row in self.rows:
            for j in row["sel"]:
                cover.append((j * MM, (j + 1) * MM))
        cover.sort()
        merged = []
        for s0, e0 in cover:
            if merged and s0 <= merged[-1][1]:
                merged[-1][1] = max(merged[-1][1], e0)
            else:
                merged.append([s0, e0])
        self.chunks = []        # (start, end) bcast tiles
        for s0, e0 in merged:
            x = s0
            while x < e0:
                self.chunks.append((x, min(x + TCH, e0)))
                x += TCH

        # ---- op list: partial-head + selective blocks + tail pieces ----
        self.ops = []           # (kind, row, (start, end) or None)
        for r, row in enumerate(self.rows):
            self.ops.append(("head", r, None))
            for j in row["sel"]:
                self.ops.append(("sel", r, (j * MM, (j + 1) * MM)))
            t0, t1 = row["tail"]
            for (cs, ce) in self.chunks:
                s0, e0 = max(t0, cs), min(t1, ce)
                if s0 < e0:
                    self.ops.append(("tail", r, (s0, e0)))

        def op_elems(i):
            kind, r, x = self.ops[i]
            return self.rows[r]["wh"] if kind == "head" else x[1] - x[0]

        self.op_elems = op_elems

        # ---- DMA schedule: interleave head segments and bcast chunks
        # round-robin over 3 rings (sync / gpsimd / tensor) ----
        self.hoff = np.concatenate(
            [[0], np.cumsum([row["wh"] for row in self.rows])]).astype(int)
        HW = int(self.hoff[-1])
        self.HW = HW
        dma_items = [("cols", None)]
        for g0 in range(0, self.nrow, 3):
            dma_items.append(("headg", (g0, min(g0 + 3, self.nrow))))
        for cc in self.chunks:
            dma_items.append(("chunk", cc))
        ring_t = [0.0, 0.0, 0.0]
        self.dma_ring = {}
        self.head_arr = [0.0] * self.nrow
        self.chunk_arr = {}
        nch = 0
        for (knd, val) in dma_items:
            if knd == "cols":
                rg, sz, rate = 0, len(self.ops) * P * 4, DMA_NAT
            elif knd == "headg":
                r0_, r1_ = val
                sz = int(self.hoff[r1_] - self.hoff[r0_]) * P * 2
                rg, rate = 0, DMA_NAT
            else:
                # broadcast chunks alternate ACT ring (fast, idle early)
                # and SP ring (after the hg segments)
                rg = 2 if nch % 2 == 0 else 0
                nch += 1
                sz, rate = (val[1] - val[0]) * P * 2, DMA_BC
            ring_t[rg] += sz * rate
            self.dma_ring[(knd, val)] = rg
            if knd == "headg":
                for rr in range(val[0], val[1]):
                    self.head_arr[rr] = ring_t[rg]
            elif knd == "chunk":
                self.chunk_arr[val] = ring_t[rg]
        self.dma_items = dma_items

        def op_arrival(i):
            kind, r, x = self.ops[i]
            if kind == "head":
                return self.head_arr[r]
            for (cs, ce) in self.chunks:
                if x[0] >= cs and x[1] <= ce:
                    return self.chunk_arr[(cs, ce)]
            raise KeyError((kind, r, x))

        # ---- lane assignment: quota split (LP balance), arrival order ----
        E = float(sum(op_elems(i) for i in range(len(self.ops))))
        rd, ra, rz = COST["dve"][0], COST["act"][0], COST["pe"][0]
        # act*x = dve: rd*y + rz*z = pe: PE_RATE*z = T ; x+y+z = E
        T = E / (1.0 / ra + (1.0 - rz / PE_RATE) / rd + 1.0 / PE_RATE)
        quota = {"act": 1.22 * T / ra, "pe": 1.25 * T / PE_RATE}
        quota["dve"] = E - quota["act"] - quota["pe"]
        order_i = sorted(range(len(self.ops)), key=op_arrival)
        used = {"dve": 0.0, "act": 0.0, "pe": 0.0}
        load = {"dve": 0.0, "act": 0.0, "pe": 0.0}
        self.lane = [None] * len(self.ops)
        for i in order_i:
            el = op_elems(i)
            arr = op_arrival(i)
            cands = ["dve"] + (["act"] if el >= ACT_MIN else []) \
                + (["pe"] if el % MM == 0 else [])
            best = max(cands, key=lambda ln: quota[ln] - used[ln])
            self.lane[i] = best
            used[best] += el
            r0, o0 = COST[best]
            if best == "pe":
                t0 = max(load["dve"], arr) + el * r0 + o0
                load["dve"] = t0
                load["pe"] = max(load["pe"], t0) + el * PE_RATE
            else:
                load[best] = max(load[best], arr) + el * r0 + o0
        self.load = load
        self.emit_order = order_i

        self._build_core_data()
        self.pattern = (self.nrow,
                        tuple((row["suffix"], row["wh"], row["tail"],
                               tuple(row["sel"])) for row in self.rows),
                        tuple(self.lane), tuple(self.chunks))

    def _build_core_data(self):
        nrow = self.nrow
        HW = self.HW
        self.slot_ev = np.full((NCORES, nrow, P), -1, np.int64)
        self.cslot = np.full((NCORES, nrow, P), -1, np.int64)
        self.hg = np.full((NCORES, P, HW), NEG16, np.float16)
        self.thr = np.full((NCORES, nrow, P), BIG32, np.float32)
        self.nmid = np.full((NCORES, nrow, P), -BIG32, np.float32)
        mids = _mid32(self.t16)
        self.Rsum = 0
        for r, row in enumerate(self.rows):
            s, e = row["s"], row["e"]
            idx = np.arange(s, e)
            if not row["suffix"]:
                self.Rsum += int(self.R[s:e].sum())
            for k in range(NCORES):
                mine = idx[k::NCORES]
                self.slot_ev[k, r, :len(mine)] = mine
                ce = self.c[mine]
                self.cslot[k, r, :len(mine)] = ce
                self.thr[k, r, :len(mine)] = self.t16[mine].astype(np.float32)
                self.nmid[k, r, :len(mine)] = -mids[mine]
                h0 = int(self.hoff[r])
                if row["suffix"]:
                    # partial head [c_e, MM*ceil(c_e/MM))
                    for p, cc in enumerate(ce):
                        a = int(-(-int(cc) // MM)) * MM
                        L = min(a, N) - int(cc)
                        if L > 0:
                            self.hg[k, p, h0:h0 + L] = \
                                self.yh16[int(cc):int(cc) + L]
                else:
                    # partial head [MM*floor(c_e/MM), c_e)
                    for p, cc in enumerate(ce):
                        b = (int(cc) // MM) * MM
                        L = int(cc) - b
                        if L > 0:
                            self.hg[k, p, h0:h0 + L] = \
                                self.yh16[b:int(cc)]

        self.op_cols = []
        for i, (kind, r, x) in enumerate(self.ops):
            ln = self.lane[i]
            base = self.nmid[:, r, :] if ln == "act" else self.thr[:, r, :]
            if kind == "sel":
                j = x[0] // MM
                cs = self.cslot[:, r, :]
                if self.rows[r]["suffix"]:
                    active = (cs >= 0) & (cs <= j * MM)
                else:
                    active = (cs >= 0) & (cs >= (j + 1) * MM)
                maskv = np.float32(-BIG32) if ln == "act" else BIG32
                col = np.where(active, base, maskv).astype(np.float32)
            else:
                col = base.astype(np.float32)
            self.op_cols.append(col)

    # ---- numpy simulation ----
    def simulate(self):
        nops = len(self.ops)
        acc = np.zeros((NCORES, P, nops), np.float64)
        for i, (kind, r, x) in enumerate(self.ops):
            ln = self.lane[i]
            h0 = int(self.hoff[r])
            for k in range(NCORES):
                if kind == "head":
                    data = self.hg[k][:, h0:h0 + self.rows[r]["wh"]]
                    data = data.astype(np.float32)
                else:
                    s0, e0 = x
                    data = np.broadcast_to(
                        self.yh16[s0:e0].astype(np.float32), (P, e0 - s0))
                col = self.op_cols[i][k][:, None]
                if ln == "act":
                    sg = np.where(data > -col, 1.0, -1.0)
                    acc[k, :, i] = sg.sum(axis=1)
                else:
                    acc[k, :, i] = (data >= col).sum(axis=1)
        return acc

    def combine_from_acc(self, accs):
        S1 = float(self.Rsum)
        for i, (kind, r, x) in enumerate(self.ops):
            L = self.rows[r]["wh"] if kind == "head" else x[1] - x[0]
            sgn = 1.0 if self.rows[r]["suffix"] else -1.0
            for k in range(NCORES):
                a = accs[k][:, i].astype(np.float64)
                if self.lane[i] == "act":
                    cntk = (L * P + a.sum()) / 2.0
                else:
                    cntk = a.sum()
                S1 += sgn * cntk
        return self._final(S1)

    def _final(self, S1):
        c_sum = np.float32(S1 - self.ns - self.OV)
        t_sum = np.float32(self.S2 - self.ns - self.OV2)
        return np.asarray(np.float32(c_sum / t_sum))

    def combine_device(self, results):
        S1 = float(self.Rsum)
        for i, (kind, r, x) in enumerate(self.ops):
            ln = self.lane[i]
            if ln == "pe":
                continue
            L = self.rows[r]["wh"] if kind == "head" else x[1] - x[0]
            sgn = 1.0 if self.rows[r]["suffix"] else -1.0
            for k in range(NCORES):
                a = results[k][f"o_acc_{ln}"][:, i].astype(np.float64)
                if ln == "act":
                    cntk = (L * P + a.sum()) / 2.0
                else:
                    cntk = a.sum()
                S1 += sgn * cntk
        for k in range(NCORES):
            ps = results[k]["o_ps"].astype(np.float64).reshape(-1)
            S1 += ps[:MM].sum() - ps[MM:].sum()
        return self._final(S1)


def build_bass(plan):
    nc = bacc.Bacc(debug=False, num_devices=NCORES)
    nrow, nops = plan.nrow, len(plan.ops)
    HW = plan.HW

    ybc = nc.dram_tensor("ybc", [1, N], FP16, kind="ExternalInput")
    hg = nc.dram_tensor("hg", [P, HW], FP16, kind="ExternalInput")
    cols = nc.dram_tensor("cols", [P, nops], FP32, kind="ExternalInput")
    o_accs = {ln: nc.dram_tensor(f"o_acc_{ln}", [P, nops], FP32,
                                 kind="ExternalOutput")
              for ln in ("dve", "act")}
    o_ps = nc.dram_tensor("o_ps", [1, 2 * MM], FP32, kind="ExternalOutput")

    n_mm = [0, 0]
    for i, (kind, r, x) in enumerate(plan.ops):
        if plan.lane[i] == "pe":
            L = plan.rows[r]["wh"] if kind == "head" else x[1] - x[0]
            n_mm[0 if plan.rows[r]["suffix"] else 1] += L // MM

    rings = {0: nc.sync, 1: nc.gpsimd, 2: nc.scalar}

    with tile.TileContext(nc) as tc:
        with (
            tc.tile_pool(name="c", bufs=1) as cpool,
            tc.tile_pool(name="ps", bufs=1, space="PSUM") as ppool,
        ):
            colt = cpool.tile([P, nops], FP32)
            hgt = cpool.tile([P, HW], FP16)
            bts = {}
            for (knd, val) in plan.dma_items:
                rg = rings[plan.dma_ring[(knd, val)]]
                if knd == "cols":
                    rg.dma_start(out=colt[:, :], in_=cols[:, :])
                elif knd == "headg":
                    h0 = int(plan.hoff[val[0]])
                    h1 = int(plan.hoff[val[1]])
                    rg.dma_start(out=hgt[:, h0:h1], in_=hg[:, h0:h1])
                else:
                    cs, ce = val
                    t = cpool.tile([P, ce - cs], FP16, name=f"b{cs}")
                    rg.dma_start(
                        out=t[:, :],
                        in_=ybc[0:1, cs:ce].to_broadcast((P, ce - cs)))
                    bts[(cs, ce)] = t

            def locate(s0, e0):
                for (bs, be), t in bts.items():
                    if s0 >= bs and e0 <= be:
                        return t[:, s0 - bs:e0 - bs]
                raise KeyError((s0, e0))

            accs = {ln: cpool.tile([P, nops], FP32, name=f"acc_{ln}")
                    for ln in ("dve", "act")}
            SW = max([TCH] + [row["wh"] for row in plan.rows])
            scratch = {ln: cpool.tile([P, SW], FP16, name=f"s_{ln}")
                       for ln in ("dve", "act")}
            zscr = [cpool.tile([P, SW], BF16, name=f"z{j}") for j in range(3)]
            ones_w = cpool.tile([P, 1], BF16)
            nc.vector.memset(ones_w[:, :], 1.0)
            banks = [ppool.tile([1, MM], FP32, name="bankS"),
                     ppool.tile([1, MM], FP32, name="bankP")]
            seen = [0, 0]
            zrot = [0]

            def emit(i):
                kind, r, x = plan.ops[i]
                ln = plan.lane[i]
                if kind == "head":
                    h0 = int(plan.hoff[r])
                    L = plan.rows[r]["wh"]
                    src = hgt[:, h0:h0 + L]
                else:
                    src = locate(x[0], x[1])
                    L = x[1] - x[0]
                if ln == "dve":
                    nc.vector.tensor_scalar(
                        out=scratch[ln][:, 0:L], in0=src,
                        scalar1=colt[:, i:i + 1], scalar2=0.0,
                        op0=Alu.is_ge, op1=Alu.add,
                        accum_out=accs[ln][:, i:i + 1])
                elif ln == "act":
                    nc.scalar.activation(
                        out=scratch[ln][:, 0:L], in_=src, func=ActF.Sign,
                        bias=colt[:, i:i + 1], scale=1.0,
                        accum_out=accs[ln][:, i:i + 1])
                else:
                    b = 0 if plan.rows[r]["suffix"] else 1
                    z = zscr[zrot[0] % 3]
                    zrot[0] += 1
                    nc.vector.tensor_scalar(
                        out=z[:, 0:L], in0=src,
                        scalar1=colt[:, i:i + 1], scalar2=None,
                        op0=Alu.is_ge)
                    for ch in range(L // MM):
                        seen[b] += 1
                        nc.tensor.matmul(
                            banks[b][0:1, 0:MM], ones_w[:, :],
                            z[:, ch * MM:(ch + 1) * MM],
                            start=(seen[b] == 1),
                            stop=(seen[b] == n_mm[b]))

            for i in plan.emit_order:
                emit(i)

            stg = cpool.tile([1, 2 * MM], FP32)
            for b in range(2):
                if n_mm[b] == 0:
                    nc.vector.memset(stg[:, b * MM:(b + 1) * MM], 0.0)
                else:
                    nc.vector.tensor_copy(out=stg[:, b * MM:(b + 1) * MM],
                                          in_=banks[b][0:1, :])
            rings[0].dma_start(out=o_ps[:, :], in_=stg[:, :])
            for ln in ("dve", "act"):
                rings[0].dma_start(out=o_accs[ln][:, :], in_=accs[ln][:, :])

    nc.compile()
    return nc


_NC_CACHE = {}


def _get_nc(plan):
    key = plan.pattern
    if key not in _NC_CACHE:
        _NC_CACHE[key] = build_bass(plan)
    return _NC_CACHE[key]


def kernel(y, y_hat, status, _run_kwargs=None, _simulate=False):
    plan = Plan(y, y_hat, status)
    if _simulate:
        acc = plan.simulate()
        return plan.combine_from_acc([acc[k] for k in range(NCORES)])
    nc = _get_nc(plan)
    ybc2 = np.ascontiguousarray(plan.yh16.reshape(1, N))
    in_maps = []
    for k in range(NCORES):
        in_maps.append({
            "ybc": ybc2,
            "hg": np.ascontiguousarray(plan.hg[k]),
            "cols": np.ascontiguousarray(
                np.stack([plan.op_cols[i][k] for i in range(len(plan.ops))],
                         axis=1)),
        })
    kw = dict(_run_kwargs or {})
    res = bass_utils.run_bass_kernel_spmd(
        nc, in_maps, core_ids=list(range(NCORES)), **kw)
    out = plan.combine_device(res.results)
    if _run_kwargs is not None:
        return out, res
    return out


if __name__ == "__main__":
    rng = np.random.default_rng(0)
    y = rng.standard_normal(N).astype(np.float32)
    yh = rng.standard_normal(N).astype(np.float32)
    st = (rng.integers(0, 2, N)).astype(np.int32)
    print(kernel(y, yh, st, _simulate=True))


# revision 29
# speedup vs baseline: 1.1348x; 1.1348x over previous
"""Concordance-index (C-index) kernel for Trainium2, 8 NeuronCores — v4.2.

Math
----
Reference computes, over all pairs i<j of N=16384 samples:
    cc = ((y_i>=y_j & yh_i>=yh_j & st_j) | (y_i<=y_j & yh_i<=yh_j & st_i)) & triu
    tp = ((y_i<=y_j & st_i) | (y_i>=y_j & st_j)) & triu
    out = sum(cc) / sum(tp)

With A_ae = [y_a >= y_e], B_ae = [yh_a >= yh_e] over (a = all, e = event):
    sum(cc) = S1 - ns - OV,   S1 = sum_e T_e,  T_e = sum_a A_ae * B_ae
    sum(tp) = S2 - ns - OV2,  S2 = sum_e sum_a A_ae
where OV / OV2 are exact-fp32-tie corrections among event pairs.

Key ideas
---------
* Sort the a-axis by fp32 y: A_ae = [a >= c_e], c_e host-exact, so S2 is
  pure host math and T_e is a 1-D count of [yh_a >= t_e] over the suffix
  [c_e, N) — or R_e (host-known total) minus the prefix count.  Each
  event pays only its SHORTER side: ~N^2/8 compares total.
* Events sorted by c, 1024 per row (128/core x 8 cores).  The ragged
  per-event boundary is absorbed into a HOST-GATHERED window per event
  (padded with -60000 -> compares contribute 0); the rest is a
  row-uniform 64-aligned tail.  Every device op is a plain fused
  compare+accumulate — no masks, no products.
* S1 only needs GLOBAL sums, so three lanes run concurrently:
  ACT (Sign+accum), DVE (is_ge+add fused), and DVE plain is_ge in 4x
  DVE perf mode feeding PE ones-matmul column reductions in PSUM.
* DMA is chunked over three queue rings (sync/gpsimd/tensor) in the
  order ops consume it; ops are emitted in estimated-arrival order.
* fp16 compare semantics host+device; only fp16-vs-fp32 tie noise
  remains (~1e-4 rel; gate 2e-2).
"""

import math
import os
import sys

import numpy as np

for _p in ("/opt/trn_rl_repo", "/root/.axon_site", "/root/.axon_site/_ro/trn_rl_repo"):
    if os.path.isdir(_p) and _p not in sys.path:
        sys.path.append(_p)

import ml_dtypes  # noqa: F401  (env check)

import concourse.bacc as bacc
import concourse.mybir as mybir
from concourse import bass_utils
from concourse import tile

N = 16384
P = 128
NCORES = 8
NEG16 = np.float16(-60000.0)
BIG32 = np.float32(1e30)
TCH = 3072          # tail/bcast chunk width
MM = 512            # PE matmul moving width (one PSUM bank)

FP32 = mybir.dt.float32
FP16 = mybir.dt.float16
BF16 = mybir.dt.bfloat16
Alu = mybir.AluOpType
ActF = mybir.ActivationFunctionType

# measured ns/elem and ns/op overhead (TRN2, aligned ops)
COST = {
    "dve": (1.09, 240.0),    # tensor_scalar is_ge+add fused accum (1x)
    "act": (0.95, 600.0),    # activation Sign with accum (big read-acc cost)
    "pe": (0.29, 130.0),     # DVE plain is_ge at 4x (feeds PE)
}
ACT_MIN = 2048               # ACT lane only for ops at least this wide
PE_RATE = 1.05               # ns/elem ones-matmul reduce incl ldweights
DMA_NAT = 6.1e-3             # ns/SBUF-byte, natural DMA (~163 GB/s, HBM-read bound)
DMA_BC = 1.9e-3              # ns/SBUF-byte, broadcast DMA (~530 GB/s)


def _mid32(t16):
    prev = np.nextafter(t16, np.float16(-np.inf))
    return (t16.astype(np.float32) + prev.astype(np.float32)) * np.float32(0.5)


class Plan:
    def __init__(self, y, yh, status):
        y32 = np.asarray(y, np.float32)
        yh32 = np.asarray(yh, np.float32)
        st = np.asarray(status)

        order = np.argsort(y32, kind="stable")
        ys = y32[order]
        self.yh16 = yh32[order].astype(np.float16)

        ev = np.nonzero(st == 1)[0]
        self.ns = int(len(ev))
        t16 = yh32[ev].astype(np.float16)
        c = np.searchsorted(ys, y32[ev], side="left").astype(np.int64)
        yh16s = np.sort(self.yh16)
        R = (N - np.searchsorted(yh16s, t16, side="left")).astype(np.int64)
        self.S2 = int((np.int64(N) - c).sum())

        yv = y32[ev]
        _, cnt = np.unique(yv, return_counts=True)
        self.OV2 = int((cnt.astype(np.int64) * (cnt - 1) // 2).sum())
        pair = np.stack([yv, yh32[ev]], 1)
        _, cnt2 = np.unique(pair, axis=0, return_counts=True)
        self.OV = int((cnt2.astype(np.int64) * (cnt2 - 1) // 2).sum())

        eo = np.argsort(c, kind="stable")
        self.c = c[eo]
        self.t16 = t16[eo]
        self.R = R[eo]

        self.nrow = max(1, math.ceil(self.ns / (P * NCORES)))
        self.rows = []
        for r in range(self.nrow):
            s, e = r * P * NCORES, min((r + 1) * P * NCORES, self.ns)
            cr = self.c[s:e]
            lo, hi = int(cr.min()), int(cr.max())
            suffix = int((np.int64(N) - cr).sum()) <= int(cr.sum())
            if suffix:
                a = min(N, ((hi + 1 + MM - 1) // MM) * MM)  # 512-aligned
                tail = (a, N)
                sel = [j for j in range((lo + MM - 1) // MM, a // MM)]
            else:
                b = (lo // MM) * MM
                tail = (0, b)
                sel = [j for j in range(b // MM, N // MM)
                       if MM * (j + 1) <= hi]
            self.rows.append({"suffix": suffix, "s": s, "e": e, "lo": lo,
                              "hi": hi, "wh": MM, "tail": tail, "sel": sel,
                              "abound": a if suffix else b})

        # ---- broadcast ranges: tails + selective blocks, merged ----
        cover = [row["tail"] for row in self.rows
                 if row["tail"][1] > row["tail"][0]]
        for row in self.rows:
            for j in row["sel"]:
                cover.append((j * MM, (j + 1) * MM))
        cover.sort()
        merged = []
        for s0, e0 in cover:
            if merged and s0 <= merged[-1][1]:
                merged[-1][1] = max(merged[-1][1], e0)
            else:
                merged.append([s0, e0])
        self.chunks = []        # (start, end) bcast tiles
        for s0, e0 in merged:
            x = s0
            while x < e0:
                self.chunks.append((x, min(x + TCH, e0)))
                x += TCH

        # ---- op list: partial-head + selective blocks + tail pieces ----
        self.ops = []           # (kind, row, (start, end) or None)
        for r, row in enumerate(self.rows):
            self.ops.append(("head", r, None))
            for j in row["sel"]:
                self.ops.append(("sel", r, (j * MM, (j + 1) * MM)))
            t0, t1 = row["tail"]
            for (cs, ce) in self.chunks:
                s0, e0 = max(t0, cs), min(t1, ce)
                if s0 < e0:
                    self.ops.append(("tail", r, (s0, e0)))

        def op_elems(i):
            kind, r, x = self.ops[i]
            return self.rows[r]["wh"] if kind == "head" else x[1] - x[0]

        self.op_elems = op_elems

        # ---- DMA schedule: interleave head segments and bcast chunks
        # round-robin over 3 rings (sync / gpsimd / tensor) ----
        self.hoff = np.concatenate(
            [[0], np.cumsum([row["wh"] for row in self.rows])]).astype(int)
        HW = int(self.hoff[-1])
        self.HW = HW
        dma_items = [("cols", None)]
        for cc in self.chunks:
            dma_items.append(("chunk", cc))
        for g0 in range(0, self.nrow, 3):
            dma_items.append(("headg", (g0, min(g0 + 3, self.nrow))))
        ring_t = [0.0, 0.0, 0.0]
        self.dma_ring = {}
        self.head_arr = [0.0] * self.nrow
        self.chunk_arr = {}
        nch = 0
        for (knd, val) in dma_items:
            if knd == "cols":
                rg, sz, rate = 0, len(self.ops) * P * 4, DMA_NAT
            elif knd == "headg":
                r0_, r1_ = val
                sz = int(self.hoff[r1_] - self.hoff[r0_]) * P * 2
                rg, rate = 0, DMA_NAT
            else:
                # broadcast chunks alternate ACT ring (fast, idle early)
                # and SP ring (after the hg segments)
                rg = 2 if nch % 2 == 0 else 0
                nch += 1
                sz, rate = (val[1] - val[0]) * P * 2, DMA_BC
            ring_t[rg] += sz * rate
            self.dma_ring[(knd, val)] = rg
            if knd == "headg":
                for rr in range(val[0], val[1]):
                    self.head_arr[rr] = ring_t[rg]
            elif knd == "chunk":
                self.chunk_arr[val] = ring_t[rg]
        self.dma_items = dma_items

        def op_arrival(i):
            kind, r, x = self.ops[i]
            if kind == "head":
                return self.head_arr[r]
            for (cs, ce) in self.chunks:
                if x[0] >= cs and x[1] <= ce:
                    return self.chunk_arr[(cs, ce)]
            raise KeyError((kind, r, x))

        # ---- lane assignment: quota split (LP balance), arrival order ----
        E = float(sum(op_elems(i) for i in range(len(self.ops))))
        rd, ra, rz = COST["dve"][0], COST["act"][0], COST["pe"][0]
        # act*x = dve: rd*y + rz*z = pe: PE_RATE*z = T ; x+y+z = E
        T = E / (1.0 / ra + (1.0 - rz / PE_RATE) / rd + 1.0 / PE_RATE)
        quota = {"act": 1.22 * T / ra, "pe": 1.15 * T / PE_RATE}
        quota["dve"] = E - quota["act"] - quota["pe"]
        order_i = sorted(range(len(self.ops)), key=op_arrival)
        used = {"dve": 0.0, "act": 0.0, "pe": 0.0}
        load = {"dve": 0.0, "act": 0.0, "pe": 0.0}
        self.lane = [None] * len(self.ops)
        for i in order_i:
            el = op_elems(i)
            arr = op_arrival(i)
            cands = ["dve"] + (["act"] if el >= ACT_MIN else []) \
                + (["pe"] if el % MM == 0 else [])
            best = max(cands, key=lambda ln: quota[ln] - used[ln])
            self.lane[i] = best
            used[best] += el
            r0, o0 = COST[best]
            if best == "pe":
                t0 = max(load["dve"], arr) + el * r0 + o0
                load["dve"] = t0
                load["pe"] = max(load["pe"], t0) + el * PE_RATE
            else:
                load[best] = max(load[best], arr) + el * r0 + o0
        self.load = load
        self.emit_order = order_i

        self._build_core_data()
        self.pattern = (self.nrow,
                        tuple((row["suffix"], row["wh"], row["tail"],
                               tuple(row["sel"])) for row in self.rows),
                        tuple(self.lane), tuple(self.chunks))

    def _build_core_data(self):
        nrow = self.nrow
        HW = self.HW
        self.slot_ev = np.full((NCORES, nrow, P), -1, np.int64)
        self.cslot = np.full((NCORES, nrow, P), -1, np.int64)
        self.hg = np.full((NCORES, P, HW), NEG16, np.float16)
        self.thr = np.full((NCORES, nrow, P), BIG32, np.float32)
        self.nmid = np.full((NCORES, nrow, P), -BIG32, np.float32)
        mids = _mid32(self.t16)
        self.Rsum = 0
        for r, row in enumerate(self.rows):
            s, e = row["s"], row["e"]
            idx = np.arange(s, e)
            if not row["suffix"]:
                self.Rsum += int(self.R[s:e].sum())
            for k in range(NCORES):
                mine = idx[k::NCORES]
                self.slot_ev[k, r, :len(mine)] = mine
                ce = self.c[mine]
                self.cslot[k, r, :len(mine)] = ce
                self.thr[k, r, :len(mine)] = self.t16[mine].astype(np.float32)
                self.nmid[k, r, :len(mine)] = -mids[mine]
                h0 = int(self.hoff[r])
                if row["suffix"]:
                    # partial head [c_e, MM*ceil(c_e/MM))
                    for p, cc in enumerate(ce):
                        a = int(-(-int(cc) // MM)) * MM
                        L = min(a, N) - int(cc)
                        if L > 0:
                            self.hg[k, p, h0:h0 + L] = \
                                self.yh16[int(cc):int(cc) + L]
                else:
                    # partial head [MM*floor(c_e/MM), c_e)
                    for p, cc in enumerate(ce):
                        b = (int(cc) // MM) * MM
                        L = int(cc) - b
                        if L > 0:
                            self.hg[k, p, h0:h0 + L] = \
                                self.yh16[b:int(cc)]

        self.op_cols = []
        for i, (kind, r, x) in enumerate(self.ops):
            ln = self.lane[i]
            base = self.nmid[:, r, :] if ln == "act" else self.thr[:, r, :]
            if kind == "sel":
                j = x[0] // MM
                cs = self.cslot[:, r, :]
                if self.rows[r]["suffix"]:
                    active = (cs >= 0) & (cs <= j * MM)
                else:
                    active = (cs >= 0) & (cs >= (j + 1) * MM)
                maskv = np.float32(-BIG32) if ln == "act" else BIG32
                col = np.where(active, base, maskv).astype(np.float32)
            else:
                col = base.astype(np.float32)
            self.op_cols.append(col)

    # ---- numpy simulation ----
    def simulate(self):
        nops = len(self.ops)
        acc = np.zeros((NCORES, P, nops), np.float64)
        for i, (kind, r, x) in enumerate(self.ops):
            ln = self.lane[i]
            h0 = int(self.hoff[r])
            for k in range(NCORES):
                if kind == "head":
                    data = self.hg[k][:, h0:h0 + self.rows[r]["wh"]]
                    data = data.astype(np.float32)
                else:
                    s0, e0 = x
                    data = np.broadcast_to(
                        self.yh16[s0:e0].astype(np.float32), (P, e0 - s0))
                col = self.op_cols[i][k][:, None]
                if ln == "act":
                    sg = np.where(data > -col, 1.0, -1.0)
                    acc[k, :, i] = sg.sum(axis=1)
                else:
                    acc[k, :, i] = (data >= col).sum(axis=1)
        return acc

    def combine_from_acc(self, accs):
        S1 = float(self.Rsum)
        for i, (kind, r, x) in enumerate(self.ops):
            L = self.rows[r]["wh"] if kind == "head" else x[1] - x[0]
            sgn = 1.0 if self.rows[r]["suffix"] else -1.0
            for k in range(NCORES):
                a = accs[k][:, i].astype(np.float64)
                if self.lane[i] == "act":
                    cntk = (L * P + a.sum()) / 2.0
                else:
                    cntk = a.sum()
                S1 += sgn * cntk
        return self._final(S1)

    def _final(self, S1):
        c_sum = np.float32(S1 - self.ns - self.OV)
        t_sum = np.float32(self.S2 - self.ns - self.OV2)
        return np.asarray(np.float32(c_sum / t_sum))

    def combine_device(self, results):
        S1 = float(self.Rsum)
        for i, (kind, r, x) in enumerate(self.ops):
            ln = self.lane[i]
            if ln == "pe":
                continue
            L = self.rows[r]["wh"] if kind == "head" else x[1] - x[0]
            sgn = 1.0 if self.rows[r]["suffix"] else -1.0
            for k in range(NCORES):
                a = results[k][f"o_acc_{ln}"][:, i].astype(np.float64)
                if ln == "act":
                    cntk = (L * P + a.sum()) / 2.0
                else:
                    cntk = a.sum()
                S1 += sgn * cntk
        for k in range(NCORES):
            ps = results[k]["o_ps"].astype(np.float64).reshape(-1)
            S1 += ps[:MM].sum() - ps[MM:].sum()
        return self._final(S1)


def build_bass(plan):
    nc = bacc.Bacc(debug=False, num_devices=NCORES)
    nrow, nops = plan.nrow, len(plan.ops)
    HW = plan.HW

    ybc = nc.dram_tensor("ybc", [1, N], FP16, kind="ExternalInput")
    hg = nc.dram_tensor("hg", [P, HW], FP16, kind="ExternalInput")
    cols = nc.dram_tensor("cols", [P, nops], FP32, kind="ExternalInput")
    o_accs = {ln: nc.dram_tensor(f"o_acc_{ln}", [P, nops], FP32,
                                 kind="ExternalOutput")
              for ln in ("dve", "act")}
    o_ps = nc.dram_tensor("o_ps", [1, 2 * MM], FP32, kind="ExternalOutput")

    n_mm = [0, 0]
    for i, (kind, r, x) in enumerate(plan.ops):
        if plan.lane[i] == "pe":
            L = plan.rows[r]["wh"] if kind == "head" else x[1] - x[0]
            n_mm[0 if plan.rows[r]["suffix"] else 1] += L // MM

    rings = {0: nc.sync, 1: nc.gpsimd, 2: nc.scalar}

    with tile.TileContext(nc) as tc:
        with (
            tc.tile_pool(name="c", bufs=1) as cpool,
            tc.tile_pool(name="ps", bufs=1, space="PSUM") as ppool,
        ):
            colt = cpool.tile([P, nops], FP32)
            hgt = cpool.tile([P, HW], FP16)
            bts = {}
            for (knd, val) in plan.dma_items:
                rg = rings[plan.dma_ring[(knd, val)]]
                if knd == "cols":
                    rg.dma_start(out=colt[:, :], in_=cols[:, :])
                elif knd == "headg":
                    h0 = int(plan.hoff[val[0]])
                    h1 = int(plan.hoff[val[1]])
                    rg.dma_start(out=hgt[:, h0:h1], in_=hg[:, h0:h1])
                else:
                    cs, ce = val
                    t = cpool.tile([P, ce - cs], FP16, name=f"b{cs}")
                    rg.dma_start(
                        out=t[:, :],
                        in_=ybc[0:1, cs:ce].to_broadcast((P, ce - cs)))
                    bts[(cs, ce)] = t

            def locate(s0, e0):
                for (bs, be), t in bts.items():
                    if s0 >= bs and e0 <= be:
                        return t[:, s0 - bs:e0 - bs]
                raise KeyError((s0, e0))

            accs = {ln: cpool.tile([P, nops], FP32, name=f"acc_{ln}")
                    for ln in ("dve", "act")}
            SW = max([TCH] + [row["wh"] for row in plan.rows])
            scratch = {ln: cpool.tile([P, SW], FP16, name=f"s_{ln}")
                       for ln in ("dve", "act")}
            zscr = [cpool.tile([P, SW], BF16, name=f"z{j}") for j in range(3)]
            ones_w = cpool.tile([P, 1], BF16)
            nc.vector.memset(ones_w[:, :], 1.0)
            banks = [ppool.tile([1, MM], FP32, name="bankS"),
                     ppool.tile([1, MM], FP32, name="bankP")]
            seen = [0, 0]
            zrot = [0]

            def emit(i):
                kind, r, x = plan.ops[i]
                ln = plan.lane[i]
                if kind == "head":
                    h0 = int(plan.hoff[r])
                    L = plan.rows[r]["wh"]
                    src = hgt[:, h0:h0 + L]
                else:
                    src = locate(x[0], x[1])
                    L = x[1] - x[0]
                if ln == "dve":
                    nc.vector.tensor_scalar(
                        out=scratch[ln][:, 0:L], in0=src,
                        scalar1=colt[:, i:i + 1], scalar2=0.0,
                        op0=Alu.is_ge, op1=Alu.add,
                        accum_out=accs[ln][:, i:i + 1])
                elif ln == "act":
                    nc.scalar.activation(
                        out=scratch[ln][:, 0:L], in_=src, func=ActF.Sign,
                        bias=colt[:, i:i + 1], scale=1.0,
                        accum_out=accs[ln][:, i:i + 1])
                else:
                    b = 0 if plan.rows[r]["suffix"] else 1
                    z = zscr[zrot[0] % 3]
                    zrot[0] += 1
                    nc.vector.tensor_scalar(
                        out=z[:, 0:L], in0=src,
                        scalar1=colt[:, i:i + 1], scalar2=None,
                        op0=Alu.is_ge)
                    for ch in range(L // MM):
                        seen[b] += 1
                        nc.tensor.matmul(
                            banks[b][0:1, 0:MM], ones_w[:, :],
                            z[:, ch * MM:(ch + 1) * MM],
                            start=(seen[b] == 1),
                            stop=(seen[b] == n_mm[b]))

            for i in plan.emit_order:
                emit(i)

            stg = cpool.tile([1, 2 * MM], FP32)
            for b in range(2):
                if n_mm[b] == 0:
                    nc.vector.memset(stg[:, b * MM:(b + 1) * MM], 0.0)
                else:
                    nc.vector.tensor_copy(out=stg[:, b * MM:(b + 1) * MM],
                                          in_=banks[b][0:1, :])
            rings[0].dma_start(out=o_ps[:, :], in_=stg[:, :])
            for ln in ("dve", "act"):
                rings[0].dma_start(out=o_accs[ln][:, :], in_=accs[ln][:, :])

    nc.compile()
    return nc


_NC_CACHE = {}


def _get_nc(plan):
    key = plan.pattern
    if key not in _NC_CACHE:
        _NC_CACHE[key] = build_bass(plan)
    return _NC_CACHE[key]


def kernel(y, y_hat, status, _run_kwargs=None, _simulate=False):
    plan = Plan(y, y_hat, status)
    if _simulate:
        acc = plan.simulate()
        return plan.combine_from_acc([acc[k] for k in range(NCORES)])
    nc = _get_nc(plan)
    ybc2 = np.ascontiguousarray(plan.yh16.reshape(1, N))
    in_maps = []
    for k in range(NCORES):
        in_maps.append({
            "ybc": ybc2,
            "hg": np.ascontiguousarray(plan.hg[k]),
            "cols": np.ascontiguousarray(
                np.stack([plan.op_cols[i][k] for i in range(len(plan.ops))],
                         axis=1)),
        })
    kw = dict(_run_kwargs or {})
    res = bass_utils.run_bass_kernel_spmd(
        nc, in_maps, core_ids=list(range(NCORES)), **kw)
    out = plan.combine_device(res.results)
    if _run_kwargs is not None:
        return out, res
    return out


if __name__ == "__main__":
    rng = np.random.default_rng(0)
    y = rng.standard_normal(N).astype(np.float32)
    yh = rng.standard_normal(N).astype(np.float32)
    st = (rng.integers(0, 2, N)).astype(np.int32)
    print(kernel(y, yh, st, _simulate=True))
